# revision 1
# baseline (speedup 1.0000x reference)
"""Bass/Tile kernel builder for the 2-layer GAT + MLP-head classifier (v2).

Math trick: with e_ij = s_i + d_j and row-softmax, the s_i term cancels:
  alpha = softmax_j(where(A==0, -1e9, s_i + d_j))
        = A_ij * u_j / sum_j(A_ij * u_j),   u = exp(d)
  Z = alpha @ H = (A @ (u*H)) / (A @ u)
so each GAT layer is one masked matmul A @ [u*H, u] -- the NxN attention
matrix is never materialized.

v2 layout/distribution:
- Host ships per-core transposed shards in bf16: XT [768, R], AT [N, R]
  (A is 0/1 so bf16 is exact; X is converted to bf16 on-chip by v1 anyway).
- A^T shard stays resident in SBUF across both layers.
- Each core builds only its own R rows of B = [u*H, u] from its own H
  shard, then AllGathers B ([N, dh+1] bf16) -- the 64-chunk masked-matmul
  loop is pure PE accumulation with a small bT DMA per chunk.
- reps: repeat the whole computation (fresh A loads included) for
  on-device delta timing.
"""

import numpy as np
import ml_dtypes

import concourse.bass as bass
import concourse.bacc as bacc
import concourse.mybir as mybir
import concourse.tile as tile

F32 = mybir.dt.float32
BF16 = mybir.dt.bfloat16
FP8 = mybir.dt.float8e4
P = 128

DIN = 768
DH1, DO1 = 64, 64
DH2, DO2 = 32, 64


def _bf16(x):
    return np.asarray(x, dtype=np.float32).astype(ml_dtypes.bfloat16)


def _f32(x):
    return np.ascontiguousarray(np.asarray(x, dtype=np.float32))


def _halves(R, step=512):
    out = []
    s = 0
    while s < R:
        e = min(s + step, R)
        out.append((s, e))
        s = e
    return out


def build_gat_nc(weights, N=8192, n_cores=8, reps=1, mm_free=512,
                 no_coll=False, stage=5, a_dtype="bf16", debug=False):
    ADT = {"bf16": BF16, "fp8": FP8}[a_dtype]
    R = N // n_cores            # rows per core
    C = N // P                  # j-chunks (contraction tiles)
    RC = R // P                 # local row-chunks per core
    H512 = _halves(R, 512)      # f32-moving matmul halves
    HM = _halves(R, mm_free)    # bf16-moving matmul splits

    nc = bacc.Bacc("TRN2", debug=debug, num_devices=n_cores,
                   target_bir_lowering=False)
    groups = [list(range(n_cores))]

    # ---------------- I/O ----------------
    xt_d = nc.declare_dram_parameter("XT", [DIN, R], BF16, isOutput=False)
    at_d = nc.declare_dram_parameter("AT", [N, R], ADT, isOutput=False)
    out_d = nc.declare_dram_parameter("out", [2, 1], F32, isOutput=True)

    # ---------------- constants (inlined into NEFF) ----------------
    w1_d = nc.inline_tensor(_bf16(weights["W1"]), "w1c")           # [768, 64]
    a1d_d = nc.inline_tensor(_bf16(weights["a1"][DH1:]).reshape(DH1, 1), "a1dc")
    w21_d = nc.inline_tensor(_f32(weights["W21"]), "w21c")         # [64, 64]
    b21_d = nc.inline_tensor(_f32(weights["b21"]).reshape(DO1, 1), "b21c")
    wg2_d = nc.inline_tensor(_f32(weights["Wg2"]), "wg2c")         # [64, 32]
    a2d_d = nc.inline_tensor(_bf16(weights["a2"][DH2:]).reshape(DH2, 1), "a2dc")
    w22_d = nc.inline_tensor(_f32(weights["W22"]), "w22c")         # [32, 64]
    b22_d = nc.inline_tensor(_f32(weights["b22"]).reshape(DO2, 1), "b22c")
    m1_d = nc.inline_tensor(_f32(weights["M1"]), "m1c")            # [64, 64]
    bm1_d = nc.inline_tensor(_f32(weights["bm1"]).reshape(64, 1), "bm1c")
    m2_d = nc.inline_tensor(_f32(weights["M2"]), "m2c")            # [64, 2]
    bm2_d = nc.inline_tensor(_f32(weights["bm2"]).reshape(2, 1), "bm2c")
    ident_d = nc.inline_tensor(
        np.eye(P, dtype=np.float32).astype(ml_dtypes.bfloat16), "identc")
    ones_d = nc.inline_tensor(np.ones((1, 64), np.float32), "onesc")

    # ---------------- internal DRAM (collective bounce) ----------------
    ag1_in = nc.dram_tensor("ag1_in", [R, DH1 + 1], BF16)
    ag1_out = nc.dram_tensor("ag1_out", [N, DH1 + 1], BF16,
                             addr_space="Shared")
    ag2_in = nc.dram_tensor("ag2_in", [R, DH2 + 1], BF16)
    ag2_out = nc.dram_tensor("ag2_out", [N, DH2 + 1], BF16,
                             addr_space="Shared")
    ar_in = nc.dram_tensor("ar_in", [64, 1], F32)
    ar_out = nc.dram_tensor("ar_out", [64, 1], F32, addr_space="Shared")

    with tile.TileContext(nc) as tc:
        with (
            tc.tile_pool(name="const", bufs=1) as cp,
            tc.tile_pool(name="persist", bufs=1) as pp,
            tc.tile_pool(name="work", bufs=1) as wp,
            tc.tile_pool(name="psum", bufs=1, space="PSUM") as sp_pool,
        ):
            # ---- load constants to SBUF (once)
            w1_sb = cp.tile([P, (DIN // P) * DH1], BF16, name="w1_sb")
            for kc in range(DIN // P):
                nc.sync.dma_start(w1_sb[:, kc * DH1:(kc + 1) * DH1],
                                  w1_d[kc * P:(kc + 1) * P, :])
            a1d_sb = cp.tile([DH1, 1], BF16, name="a1d_sb")
            nc.sync.dma_start(a1d_sb, a1d_d[:])
            a2d_sb = cp.tile([DH2, 1], BF16, name="a2d_sb")
            nc.sync.dma_start(a2d_sb, a2d_d[:])
            w21_sb = cp.tile([DH1, DO1], F32, name="w21_sb")
            nc.sync.dma_start(w21_sb, w21_d[:])
            b21_sb = cp.tile([DO1, 1], F32, name="b21_sb")
            nc.sync.dma_start(b21_sb, b21_d[:])
            wg2_sb = cp.tile([DO1, DH2], F32, name="wg2_sb")
            nc.sync.dma_start(wg2_sb, wg2_d[:])
            w22_sb = cp.tile([DH2, DO2], F32, name="w22_sb")
            nc.sync.dma_start(w22_sb, w22_d[:])
            b22_sb = cp.tile([DO2, 1], F32, name="b22_sb")
            nc.sync.dma_start(b22_sb, b22_d[:])
            m1_sb = cp.tile([64, 64], F32, name="m1_sb")
            nc.sync.dma_start(m1_sb, m1_d[:])
            bm1_sb = cp.tile([64, 1], F32, name="bm1_sb")
            nc.sync.dma_start(bm1_sb, bm1_d[:])
            m2_sb = cp.tile([64, 2], F32, name="m2_sb")
            nc.sync.dma_start(m2_sb, m2_d[:])
            bm2_sb = cp.tile([2, 1], F32, name="bm2_sb")
            nc.sync.dma_start(bm2_sb, bm2_d[:])
            ident_sb = cp.tile([P, P], BF16, name="ident_sb")
            nc.sync.dma_start(ident_sb, ident_d[:])
            ones_sb = cp.tile([1, 64], F32, name="ones_sb")
            nc.sync.dma_start(ones_sb, ones_d[:])

            # ---- resident A^T shard (per chunk), reloaded per rep
            abf = [pp.tile([P, R], ADT, name=f"abf{c}") for c in range(C)]

            # stage-timing scaffolding: tiny accumulator chain that keeps
            # truncated-stage variants honest (anti-DCE, serializes reps)
            f_acc = pp.tile([1, 1], F32, name="f_acc")
            nc.vector.memset(f_acc, 0.0)

            def consume(src, t):
                g = wp.tile([1, 1], F32, tag="csm", bufs=2, name=f"csm{t}")
                nc.vector.tensor_copy(g, src)
                nc.vector.tensor_tensor(f_acc, g, f_acc,
                                        op=mybir.AluOpType.add)

            def build_own_b(ht_sb, dh, a_sb, ag_in, tag):
                """Write this core's R rows of B = [u*H, u] to ag_in."""
                for lc in range(RC):
                    hTs = ht_sb[:, lc * P:(lc + 1) * P]
                    ps_tr = sp_pool.tile([P, dh], BF16, tag="ps_tr", bufs=2,
                                         name=f"ps_tr{tag}_{lc}")
                    nc.tensor.transpose(ps_tr, hTs, ident_sb[:dh, :dh])
                    ps_d = sp_pool.tile([P, 1], F32, tag="ps_d", bufs=2,
                                        name=f"ps_d{tag}_{lc}")
                    nc.tensor.matmul(ps_d, hTs, a_sb, start=True, stop=True)
                    u = wp.tile([P, 1], F32, tag="u", bufs=3,
                                name=f"u{tag}_{lc}")
                    nc.scalar.activation(u, ps_d,
                                         mybir.ActivationFunctionType.Exp)
                    bch = wp.tile([P, dh + 1], BF16, tag="bch", bufs=3,
                                  name=f"bch{tag}_{lc}")
                    nc.vector.tensor_scalar_mul(bch[:, 0:dh], ps_tr, u)
                    nc.vector.tensor_copy(bch[:, dh:dh + 1], u)
                    nc.sync.dma_start(ag_in[lc * P:(lc + 1) * P, :], bch)

            def epilogue(ps_o, dh, do, w2_sb, b2_sb, tag):
                """out_t = elu((numer/denom) @ W2 + b2), transposed [do, R]."""
                o_t = wp.tile([dh + 1, R], F32, tag="o_t", bufs=1,
                              name=f"o_t{tag}")
                nc.scalar.copy(o_t, ps_o)
                rec = wp.tile([1, R], F32, tag="rec", bufs=1, name=f"rec{tag}")
                nc.vector.reciprocal(rec, o_t[dh:dh + 1, :])
                bc_sb = wp.tile([do, R], F32, tag="bc", bufs=1,
                                name=f"bc{tag}")
                zt = wp.tile([do, R], F32, tag="ep", bufs=4, name=f"zt{tag}")
                for h, (s, e) in enumerate(H512):
                    ps_bc = sp_pool.tile([do, e - s], F32, tag="ps_ep", bufs=2,
                                         name=f"ps_bc{tag}_{h}")
                    nc.tensor.matmul(ps_bc, ones_sb[:, :do], rec[:, s:e],
                                     start=True, stop=True)
                    nc.vector.tensor_copy(bc_sb[:, s:e], ps_bc)
                    ps_p = sp_pool.tile([do, e - s], F32, tag="ps_ep", bufs=2,
                                        name=f"ps_p{tag}_{h}")
                    nc.tensor.matmul(ps_p, w2_sb, o_t[0:dh, s:e],
                                     start=True, stop=True)
                    nc.vector.tensor_tensor(zt[:, s:e], ps_p, bc_sb[:, s:e],
                                            op=mybir.AluOpType.mult)
                v = wp.tile([do, R], F32, tag="ep", bufs=4, name=f"v{tag}")
                nc.scalar.activation(v, zt,
                                     mybir.ActivationFunctionType.Identity,
                                     bias=b2_sb)
                nm = wp.tile([do, R], F32, tag="ep", bufs=4, name=f"nm{tag}")
                nc.vector.tensor_scalar_min(nm, v, 0.0)
                en = wp.tile([do, R], F32, tag="ep", bufs=4, name=f"en{tag}")
                nc.scalar.activation(en, nm, mybir.ActivationFunctionType.Exp)
                r = wp.tile([do, R], F32, tag="ep", bufs=4, name=f"r{tag}")
                nc.vector.tensor_scalar_max(r, v, 0.0)
                out_t = wp.tile([do, R], F32, tag="out_t", bufs=1,
                                name=f"out_t{tag}")
                nc.vector.scalar_tensor_tensor(
                    out_t, in0=r, scalar=-1.0, in1=en,
                    op0=mybir.AluOpType.add, op1=mybir.AluOpType.add)
                return out_t

            def masked_accum(ps_o, ag_out, dh, tag):
                """ps_o[dh+1, R] += B_c^T @ A_c over all j-chunks."""
                for c in range(C):
                    bT = wp.tile([P, dh + 1], BF16, tag="bT", bufs=6,
                                 name=f"bT{tag}_{c}")
                    nc.sync.dma_start(bT, ag_out[c * P:(c + 1) * P, :])
                    for h, (s, e) in enumerate(HM):
                        nc.tensor.matmul(ps_o[:, s:e], bT, abf[c][:, s:e],
                                         start=(c == 0), stop=(c == C - 1))

            for rep in range(reps):
                rr = f"r{rep}"

                # ---- A^T shard loads (fresh per rep, overlap everything)
                for c in range(C):
                    nc.sync.dma_start(abf[c], at_d[c * P:(c + 1) * P, :])
                if stage <= 1:
                    consume(abf[C - 1][0:1, 0:1], "s1" + rr)
                    continue

                # ---- H1^T = W1^T @ X^T
                ps_h1 = sp_pool.tile([DH1, R], F32, tag="big", bufs=1,
                                     name=f"ps_h1{rr}")
                for kc in range(DIN // P):
                    xt_sb = wp.tile([P, R], BF16, tag="xt", bufs=3,
                                    name=f"xt{kc}{rr}")
                    nc.sync.dma_start(xt_sb, xt_d[kc * P:(kc + 1) * P, :])
                    for h, (s, e) in enumerate(HM):
                        nc.tensor.matmul(ps_h1[:, s:e],
                                         w1_sb[:, kc * DH1:(kc + 1) * DH1],
                                         xt_sb[:, s:e],
                                         start=(kc == 0),
                                         stop=(kc == DIN // P - 1))
                h1t_sb = wp.tile([DH1, R], BF16, tag="ht", bufs=1,
                                 name=f"h1t{rr}")
                nc.scalar.copy(h1t_sb, ps_h1)

                # ---- layer 1: own-B build, gather, masked matmul
                build_own_b(h1t_sb, DH1, a1d_sb, ag1_in, "b1" + rr)
                nc.gpsimd.collective_compute(
                    "AllGather", mybir.AluOpType.bypass, replica_groups=groups,
                    ins=[ag1_in[:]], outs=[ag1_out[:]])
                if stage <= 2:
                    gt = wp.tile([1, 1], BF16, tag="gt", bufs=2,
                                 name=f"gt{rr}")
                    nc.sync.dma_start(gt, ag1_out[N - 1:N, 0:1])
                    consume(gt, "s2" + rr)
                    continue
                ps_o1 = sp_pool.tile([DH1 + 1, R], F32, tag="big", bufs=1,
                                     name=f"ps_o1{rr}")
                masked_accum(ps_o1, ag1_out, DH1, "1" + rr)
                out1t = epilogue(ps_o1, DH1, DO1, w21_sb, b21_sb, "1" + rr)
                if stage <= 3:
                    consume(out1t[0:1, 0:1], "s3" + rr)
                    continue

                # ---- layer 2
                ps_h2 = sp_pool.tile([DH2, R], F32, tag="big", bufs=1,
                                     name=f"ps_h2{rr}")
                for h, (s, e) in enumerate(H512):
                    nc.tensor.matmul(ps_h2[:, s:e], wg2_sb, out1t[:, s:e],
                                     start=True, stop=True)
                h2t_sb = wp.tile([DH2, R], BF16, tag="ht2", bufs=1,
                                 name=f"h2t{rr}")
                nc.scalar.copy(h2t_sb, ps_h2)

                build_own_b(h2t_sb, DH2, a2d_sb, ag2_in, "b2" + rr)
                nc.gpsimd.collective_compute(
                    "AllGather", mybir.AluOpType.bypass, replica_groups=groups,
                    ins=[ag2_in[:]], outs=[ag2_out[:]])
                ps_o2 = sp_pool.tile([DH2 + 1, R], F32, tag="big", bufs=1,
                                     name=f"ps_o2{rr}")
                masked_accum(ps_o2, ag2_out, DH2, "2" + rr)
                out2t = epilogue(ps_o2, DH2, DO2, w22_sb, b22_sb, "2" + rr)
                if stage <= 4:
                    consume(out2t[0:1, 0:1], "s4" + rr)
                    continue

                # ---- head: mean over all rows -> MLP -> [2]
                gsum = wp.tile([64, 1], F32, tag="gsum", bufs=1,
                               name=f"gsum{rr}")
                nc.vector.reduce_sum(gsum, out2t, axis=mybir.AxisListType.X)
                nc.sync.dma_start(ar_in[:], gsum)
                nc.gpsimd.collective_compute(
                    "AllReduce", mybir.AluOpType.add, replica_groups=groups,
                    ins=[ar_in[:]], outs=[ar_out[:]])
                g_sb = wp.tile([64, 1], F32, tag="gsum", bufs=1,
                               name=f"g{rr}")
                nc.sync.dma_start(g_sb, ar_out[:])
                ps_hd = sp_pool.tile([64, 1], F32, tag="ps_ep", bufs=2,
                                     name=f"ps_hd{rr}")
                nc.tensor.matmul(ps_hd, m1_sb, g_sb, start=True, stop=True)
                h_sb = wp.tile([64, 1], F32, tag="gsum", bufs=1,
                               name=f"h{rr}")
                nc.scalar.activation(h_sb, ps_hd,
                                     mybir.ActivationFunctionType.Relu,
                                     bias=bm1_sb, scale=1.0 / N)
                ps_f = sp_pool.tile([2, 1], F32, tag="ps_ep", bufs=2,
                                    name=f"ps_f{rr}")
                nc.tensor.matmul(ps_f, m2_sb, h_sb, start=True, stop=True)
                f_sb = wp.tile([2, 1], F32, tag="gsum", bufs=1,
                               name=f"f{rr}")
                nc.scalar.activation(f_sb, ps_f,
                                     mybir.ActivationFunctionType.Identity,
                                     bias=bm2_sb)
                nc.sync.dma_start(out_d[:], f_sb)

            if stage <= 4:
                nc.sync.dma_start(out_d[0:1, :], f_acc)

    return nc


def numpy_reference(X, A, w):
    """Straight fp32 numpy port of the jax reference (for small-N checks)."""
    def softmax(e):
        m = e.max(axis=1, keepdims=True)
        x = np.exp(e - m)
        return x / x.sum(axis=1, keepdims=True)

    def gat(Xl, W, a, W2, b2):
        H = Xl @ W
        dh = W.shape[1]
        e = (H @ a[:dh])[:, None] + (H @ a[dh:])[None, :]
        e = np.where(A == 0, np.float32(-1e9), e).astype(np.float32)
        alpha = softmax(e)
        Z = alpha @ H
        zz = Z @ W2 + b2
        return np.where(zz > 0, zz, np.exp(np.minimum(zz, 0)) - 1)

    Z = gat(X, w["W1"], w["a1"], w["W21"], w["b21"])
    Z = gat(Z, w["Wg2"], w["a2"], w["W22"], w["b22"])
    g = Z.mean(axis=0)
    h = np.maximum(g @ w["M1"] + w["bm1"], 0)
    return h @ w["M2"] + w["bm2"]


def make_in_maps(X, A, N, n_cores, a_dtype="bf16"):
    """Host-side sharding: per-core transposed shards (A exact in either)."""
    R = N // n_cores
    adt = {"bf16": ml_dtypes.bfloat16, "fp8": ml_dtypes.float8_e4m3}[a_dtype]
    Xb = np.asarray(X, np.float32).astype(ml_dtypes.bfloat16)
    Ab = np.asarray(A, np.float32).astype(adt)
    maps = []
    for k in range(n_cores):
        maps.append({
            "XT": np.ascontiguousarray(Xb[k * R:(k + 1) * R, :].T),
            "AT": np.ascontiguousarray(Ab[k * R:(k + 1) * R, :].T),
        })
    return maps


# =====================================================================
# Harness entry point: full inputs in, full output out.
# =====================================================================

_KERNEL_STATE = {}


def kernel(**inputs):
    import numpy as np
    from concourse.bass_utils import run_bass_kernel_spmd

    N = inputs["A"].shape[0]
    n_cores = 8
    w = {k: np.asarray(inputs[k]) for k in
         ("W1", "a1", "W21", "b21", "Wg2", "a2", "W22", "b22",
          "M1", "bm1", "M2", "bm2")}
    nc = build_gat_nc(w, N=N, n_cores=n_cores, reps=1, debug=False)
    nc.finalize()
    in_maps = make_in_maps(np.asarray(inputs["X"]), np.asarray(inputs["A"]),
                           N, n_cores)
    res = run_bass_kernel_spmd(nc, in_maps, core_ids=list(range(n_cores)),
                               trace=False)
    _KERNEL_STATE.update(results=res, nc=nc, in_maps=in_maps, w=w, N=N)
    return np.asarray(res.results[0]["out"]).reshape(2).astype(np.float32)

